# revision 2
# baseline (speedup 1.0000x reference)
"""Trainium2 Bass kernel for LiftSplatShoot voxel pooling (segment_reduce).

kernel(**inputs) takes the FULL inputs and returns the FULL output
(B, NZ*C, NY, NX) float32.

Strategy (8 NeuronCores = 4 batches x 2 BEV-grid halves, fully disjoint):
  host: replicate the reference geometry with eager jnp ops (bit-identical
        voxel assignment), sort each core's points by dense output row, pad
        every voxel run to a multiple of 16 ("groups"), pack voxel-atomic
        chunks of <=128 groups, and pre-gather x into the device layout.
  device (SPMD, per chunk):
        DMA x tile [128 groups, 16*64] -> DVE tree-folds the 16 members ->
        group sums [128,64]; onehot(group_slot)=is_equal(slot, iota) ->
        PE matmul segment-reduce into PSUM [128 slots, 64] -> staged in SBUF;
        every S chunks one dma_scatter_add accumulates the staged slot sums
        into their dense rows (outputs are zero-initialized by the runtime;
        unused slots add +0.0 into a known-empty row).
  host: concatenate the 8 disjoint dense sub-grids and transpose to
        (B, C, NY, NX).
"""
import numpy as np

# ---- static problem config (hardcoded per contest rules) ----
B, N, C, D = 4, 4, 64, 41
OGH, OGW, DS = 256, 704, 16
FH, FW = OGH // DS, OGW // DS  # 16, 44
XB = (-51.2, 51.2, 0.4)
YB = (-51.2, 51.2, 0.4)
ZB = (-10.0, 10.0, 20.0)
NX, NY, NZ = 256, 256, 1
NP = B * N * D * FH * FW

CH = 64     # channels per point row
G = 16      # members per group
VC = NZ * NY * NX // 2  # dense rows per core (half a batch grid) = 32768
NGC = 30    # chunks per core (max over cores is 30; others padded)
S = 6       # chunks per scatter instruction
NSCAT = NGC // S
TOK = S * 128

_CACHE = {}


def _geometry_rows(rots, trans, intrins, post_rots, post_trans):
    """Replicate reference geometry exactly (same eager jnp ops) and return
    the global flat voxel index per point and the kept mask (numpy).

    Runs on the jax CPU backend: the axon/neuron backend cannot lower
    jnp.linalg.inv (triangular-solve unsupported), and the grading reference
    must therefore run on CPU as well — matching its numerics bit-for-bit.
    """
    import jax
    import jax.numpy as jnp
    cpu = jax.local_devices(backend="cpu")[0]
    with jax.default_device(cpu):
        return _geometry_rows_impl(jnp, rots, trans, intrins, post_rots,
                                   post_trans)


def _geometry_rows_impl(jnp, rots, trans, intrins, post_rots, post_trans):
    rots = jnp.asarray(rots)
    trans = jnp.asarray(trans)
    intrins = jnp.asarray(intrins)
    post_rots = jnp.asarray(post_rots)
    post_trans = jnp.asarray(post_trans)

    dx = jnp.array([XB[2], YB[2], ZB[2]], jnp.float32)
    bx = jnp.array([XB[0] + XB[2] / 2.0, YB[0] + YB[2] / 2.0,
                    ZB[0] + ZB[2] / 2.0], jnp.float32)
    ds = (2.0 + jnp.arange(D, dtype=jnp.float32)).reshape(D, 1, 1) \
        * jnp.ones((1, FH, FW), jnp.float32)
    xs = jnp.linspace(0.0, OGW - 1, FW, dtype=jnp.float32).reshape(1, 1, FW) \
        * jnp.ones((D, FH, 1), jnp.float32)
    ys = jnp.linspace(0.0, OGH - 1, FH, dtype=jnp.float32).reshape(1, FH, 1) \
        * jnp.ones((D, 1, FW), jnp.float32)
    frustum = jnp.stack([xs, ys, ds], -1)

    pts = frustum[None, None] - post_trans[:, :, None, None, None, :]
    pts = jnp.einsum('bnij,bndhwj->bndhwi', jnp.linalg.inv(post_rots), pts)
    pts = jnp.concatenate([pts[..., :2] * pts[..., 2:3], pts[..., 2:3]], -1)
    combine = rots @ jnp.linalg.inv(intrins)
    geom = jnp.einsum('bnij,bndhwj->bndhwi', combine, pts) \
        + trans[:, :, None, None, None, :]

    vox = jnp.floor((geom.reshape(NP, 3) - (bx - dx / 2.0)) / dx).astype(jnp.int32)
    vox = np.asarray(vox)
    kept = (vox[:, 0] >= 0) & (vox[:, 0] < NX) & (vox[:, 1] >= 0) \
        & (vox[:, 1] < NY) & (vox[:, 2] >= 0) & (vox[:, 2] < NZ)
    bix = np.repeat(np.arange(B, dtype=np.int64), NP // B)
    flat = ((bix * NZ + vox[:, 2].astype(np.int64)) * NY + vox[:, 1]) * NX + vox[:, 0]
    return flat, kept


def _build_kernel():
    import concourse.bacc as bacc
    import concourse.mybir as mybir
    import concourse.tile as tile
    F32 = mybir.dt.float32
    I16 = mybir.dt.int16

    nc = bacc.Bacc("TRN2", target_bir_lowering=False, debug=False,
                   num_devices=8)
    xd = nc.dram_tensor("xd", [NGC, 128, G * CH], F32, kind="ExternalInput")
    gslots = nc.dram_tensor("gslots", [NGC, 128, 1], F32, kind="ExternalInput")
    idxs = nc.dram_tensor("idxs", [NSCAT, 128, TOK // 16], I16,
                          kind="ExternalInput")
    out = nc.dram_tensor("out", [VC, CH], F32, kind="ExternalOutput")
    with tile.TileContext(nc) as tc:
        with (
            tc.tile_pool(name="const", bufs=1) as cp,
            tc.tile_pool(name="xp", bufs=4) as xpool,
            tc.tile_pool(name="ohp", bufs=3) as ohpool,
            tc.tile_pool(name="ps2", bufs=3, space="PSUM") as ps2pool,
            tc.tile_pool(name="stg", bufs=2) as stgpool,
            tc.tile_pool(name="idxp", bufs=2) as idxpool,
            tc.tile_pool(name="gsum", bufs=4) as gsumpool,
        ):
            iota_t = cp.tile([128, 128], F32)
            nc.gpsimd.iota(iota_t[:], pattern=[[1, 128]], base=0,
                           channel_multiplier=0,
                           allow_small_or_imprecise_dtypes=True)
            gs_all = cp.tile([128, NGC], F32)
            nc.sync.dma_start(out=gs_all[:],
                              in_=gslots[:].rearrange("k p one -> p (k one)"))
            for g in range(NSCAT):
                idx_t = idxpool.tile([128, TOK // 16], I16)
                nc.sync.dma_start(out=idx_t[:], in_=idxs[g])
                stage_t = stgpool.tile([128, S, CH], F32)
                for kl in range(S):
                    k = g * S + kl
                    x_t = xpool.tile([128, G * CH], F32)
                    nc.sync.dma_start(out=x_t[:], in_=xd[k])
                    v = x_t
                    nc.vector.tensor_add(out=v[:, 0:8 * CH], in0=v[:, 0:8 * CH],
                                         in1=v[:, 8 * CH:16 * CH])
                    nc.vector.tensor_add(out=v[:, 0:4 * CH], in0=v[:, 0:4 * CH],
                                         in1=v[:, 4 * CH:8 * CH])
                    nc.vector.tensor_add(out=v[:, 0:2 * CH], in0=v[:, 0:2 * CH],
                                         in1=v[:, 2 * CH:4 * CH])
                    gsum_t = gsumpool.tile([128, CH], F32)
                    nc.vector.tensor_add(out=gsum_t[:], in0=v[:, 0:CH],
                                         in1=v[:, CH:2 * CH])
                    oh_t = ohpool.tile([128, 128], F32)
                    nc.vector.tensor_tensor(
                        out=oh_t[:],
                        in0=gs_all[:, k:k + 1].to_broadcast([128, 128]),
                        in1=iota_t[:], op=mybir.AluOpType.is_equal)
                    ps2_t = ps2pool.tile([128, CH], F32)
                    nc.tensor.matmul(out=ps2_t[:], lhsT=oh_t[:], rhs=gsum_t[:],
                                     start=True, stop=True)
                    nc.vector.tensor_copy(out=stage_t[:, kl, :], in_=ps2_t[:])
                nc.gpsimd.dma_scatter_add(out[:], stage_t[:], idx_t[:], TOK,
                                          TOK, CH)
    nc.finalize()
    return nc


def _plan_core(rows, order):
    """rows ascending (local dense rows in [0, VC)); order: matching global
    point indices."""
    uniq, counts = np.unique(rows, return_counts=True)
    used = set(uniq.tolist())
    dump = next(r for r in range(VC) if r not in used)

    chunks = []
    cur, cur_groups = [], 0
    pos = 0
    for r, c in zip(uniq.tolist(), counts.tolist()):
        ng = -(-c // G)
        assert ng <= 128, f"voxel run {c} needs {ng} groups"
        if cur_groups + ng > 128:
            chunks.append(cur)
            cur, cur_groups = [], 0
        cur.append((r, pos, c, ng))
        cur_groups += ng
        pos += c
    if cur:
        chunks.append(cur)
    nck = len(chunks)
    assert nck <= NGC, f"core needs {nck} chunks > NGC={NGC}"

    gslot = np.zeros((NGC, 128), np.float32)
    chunk_rows = np.full((NGC, 128), dump, np.int32)
    gather = np.full((NGC, 128, G), -1, np.int64)
    for k, ch in enumerate(chunks):
        gi = 0
        for si, (r, start, cnt, ng) in enumerate(ch):
            chunk_rows[k, si] = r
            for j in range(ng):
                lo = start + j * G
                hi = start + min((j + 1) * G, cnt)
                gslot[k, gi] = si
                gather[k, gi, :hi - lo] = order[lo:hi]
                gi += 1
        assert gi <= 128
    return gslot, chunk_rows, gather


def _core_inputs(gslot, chunk_rows, gather, xf_ext):
    gidx = gather.copy()
    gidx[gidx < 0] = xf_ext.shape[0] - 1
    xd = xf_ext[gidx.reshape(-1)].reshape(NGC, 128, G * CH)

    idx_tok = chunk_rows.reshape(NSCAT, TOK)
    idxs16 = np.zeros((NSCAT, 16, TOK // 16), np.int16)
    t = np.arange(TOK)
    idxs16[:, t % 16, t // 16] = idx_tok.astype(np.int16)
    idxs = np.tile(idxs16, (1, 8, 1))
    return dict(xd=np.ascontiguousarray(xd),
                gslots=np.ascontiguousarray(gslot[:, :, None]),
                idxs=np.ascontiguousarray(idxs))


def kernel(x, rots, trans, intrins, post_rots, post_trans):
    from concourse.bass_utils import run_bass_kernel_spmd

    x = np.asarray(x, dtype=np.float32)
    flat, kept = _geometry_rows(rots, trans, intrins, post_rots, post_trans)

    xf = x.reshape(NP, CH)
    xf_ext = np.concatenate([xf, np.zeros((1, CH), np.float32)], axis=0)

    in_maps = []
    for core in range(8):
        b, half = core // 2, core % 2
        lo = b * (NZ * NY * NX) + half * VC
        m = kept & (flat >= lo) & (flat < lo + VC)
        local = (flat[m] - lo).astype(np.int64)
        order = np.nonzero(m)[0]
        srt = np.argsort(local, kind="stable")
        gslot, chunk_rows, gather = _plan_core(local[srt], order[srt])
        in_maps.append(_core_inputs(gslot, chunk_rows, gather, xf_ext))

    if "nc" not in _CACHE:
        _CACHE["nc"] = _build_kernel()
    nc = _CACHE["nc"]

    res = run_bass_kernel_spmd(nc, in_maps, core_ids=list(range(8)))

    final = np.empty((B, NZ * C, NY, NX), np.float32)
    for core in range(8):
        b, half = core // 2, core % 2
        o = np.asarray(res.results[core]["out"])  # (VC, CH)
        o = o.reshape(NY // 2, NX, CH).transpose(2, 0, 1)  # (CH, 128, 256)
        final[b, :, half * (NY // 2):(half + 1) * (NY // 2), :] = o
    return final
